# revision 31
# baseline (speedup 1.0000x reference)
# Contrastive (NT-Xent / SimCLR) loss kernel for Trainium2, 8 NeuronCores.
#
# Reference computation (N=4096, D=128, T=0.1, M=2N=8192):
#   z  = concat(z1, z2)                      [M, D]
#   zn = z / max(||z||, 1e-8)                row-normalized
#   sim = (zn @ zn.T) / T                    [M, M]
#   pos_r = 2*sim[r, partner(r)]             partner(r) = r+N mod M
#   loss = mean_r( LSE(logits_r) - pos_r ) / M
#
# v3 — symmetric "triangle via rotation" kernel.
#
# sim is symmetric, so each off-diagonal 128x128 block only needs to be
# exp'ed ONCE: its row sums serve the block's rows, and its column sums
# (partition-axis sums via ones-vector matmuls on the PE) serve the
# transposed block's rows.  This halves the dominant Scalar-engine exp
# work versus the v2 full-slab kernel (8.4M -> 4.3M exps per core).
#
# Block tiling: 64 row/col tiles of 128.  The SPMD program is identical
# on all cores; core c receives z ROTATED by 8c tiles (host-side gather).
# The program, in its rotated frame, loads tiles 0..39 and computes for
# row tiles i = 0..7:
#   - strip i: blocks (i, i..i+31):  G = znT_i^T znT_window on PE,
#     estrip = exp(10G-10) (bf16->SBUF) on ACT, row sums via one DVE
#     tensor_scalar accumulate over the strip, column sums of tiles
#     i+1..i+31 via ones-matmuls.
#   - d32 block (i, i+32): exp'd on BOTH owning cores (row sums only).
#   - praw_i = rowdot(zn_i, zn_{i+32})  (the positive-pair cosines).
# Union over the 8 rotations covers each unordered tile pair {A, B} with
# diff d = B-A mod 64: d in 1..31 exactly once, d = 32 twice (both
# orientations, row sums only, no ones -> no double count), d = 0 once.
#
# Column-sum plumbing: matmul output base partition is restricted to
# {0,32,64} and PSUM has no DMA/GpSimd route, so [1,512] ones results are
# expensive to evacuate.  Strips are therefore processed in PAIRS (i,
# i+4) whose ones windows share a 512-aligned column grid (offset by
# exactly one 512 chunk): both strips accumulate into one persistent
# 3-bank PSUM grid of 9 [1,512] slots (3 per bank at partitions
# 0/32/64), relying on per-element has_written semantics (start=True
# only on the first matmul per bank, everything else accumulates or
# first-touch-overwrites).  One DVE copy + one SWDGE DMA exports the
# grid per pair.  The host combines row/column-sum partials across cores
# and finishes the log-sum-exp + mean in float64 (O(M) work).
#
# Toolchain notes inherited from v2: this walrus rejects >1 sync wait per
# instruction, so sacrificial 1x1 ldweights (PE) / tiny scalar.mul (ACT) /
# tiny memset (DVE) absorb cross-engine waits, and the Tile kernel-tail
# drain is re-emitted as one single-wait drain per proc.

import numpy as np

import concourse.bass as bass
import concourse.mybir as mybir
import concourse.tile as tile
from concourse.tile import add_dep_helper
from contextlib import ExitStack

from concourse.bass_utils import run_bass_kernel_spmd
from concourse.masks import make_identity
from concourse.vector_clock import ScopedClock, VectorClock


def _split_drain_and_barrier(self, tick_clock, wait_clock):
    """Replacement for TileContext._drain_and_barrier: the stock version
    emits ONE drain carrying a wait for every live proc, which this walrus
    build rejects ("Too many sync wait commands"). Emit one single-wait
    drain per proc instead, then the normal barrier/cleanup."""
    nc = self.nc
    ticks = list(tick_clock.global_clock)
    for proc, t in enumerate(ticks):
        if t <= 0:
            continue
        d = nc.sync.drain()
        single = VectorClock()
        single.require_at_least(proc, t)
        wait_clock.add_sem_waits(d.ins, ScopedClock({None: single}))
    nc.all_engine_barrier()
    assert self.sems is not None
    popped = nc._tile_sem_poison_stack.pop()
    assert popped is self._sem_poison
    nc.clear_and_free_semaphores(list(self.sems.allocated().values()))
    nc.all_engine_barrier()


tile.TileContext._drain_and_barrier = _split_drain_and_barrier

F32 = mybir.dt.float32
BF16 = mybir.dt.bfloat16
AF = mybir.ActivationFunctionType
ALU = mybir.AluOpType
AX = mybir.AxisListType

N_CORES = 8
N = 4096
D = 128
M2 = 2 * N                 # 8192 rows total
T64 = M2 // 128            # 64 row/col tiles
RT = 8                     # program row tiles (strips) per core
WT = 32                    # window tiles per strip (incl. diagonal tile)
LT = RT + WT               # 40 tiles of z loaded per core
SW = WT * 128              # 4096 strip width in columns
OW = (WT - 1) * 128        # 3968 ones (column-sum) width per strip
GW = 9 * 512               # 4608 grid width (9 slots) per strip pair
GV = OW + 512              # 4480 valid grid columns per pair
NP = 5                     # phase-1 pairs of z tiles (8 tiles each)

TEMP_INV = 10.0            # 1/T
LSE_SHIFT = 10.0           # constant max-shift for the log-sum-exp

CHW = 1024                 # G chunk width (2 PSUM banks)
NCH = SW // CHW            # 4 chunks per strip
STRIP_ORDER = (0, 4, 1, 5, 2, 6, 3, 7)


def build_kernel() -> bass.Bass:
    nc = bass.Bass()

    # Constants built BEFORE the TileContext, covered by a barrier: readers
    # then carry no tracked dependency on them (deps on ancient instructions
    # materialize as spurious un-elidable semaphore waits once the sem
    # window slides past them).
    _ident_t = nc.alloc_sbuf_tensor("c_ident", [128, 128], BF16)
    make_identity(nc, _ident_t.ap())
    _ones_t = nc.alloc_sbuf_tensor("c_ones", [128, 1], BF16)
    nc.gpsimd.memset(_ones_t.ap(), 1.0)
    _ldw_t = nc.alloc_sbuf_tensor("c_ldw", [1, 1], BF16)
    nc.gpsimd.memset(_ldw_t.ap(), 0.0)
    _neg_t = nc.alloc_sbuf_tensor("c_neg", [128, 1], F32)
    nc.gpsimd.memset(_neg_t.ap(), -LSE_SHIFT)
    nc.all_engine_barrier()

    z_win = nc.dram_tensor("z_win", [LT * 128, D], F32, kind="ExternalInput")
    out_rs = nc.dram_tensor("out_rs", [128, RT], F32, kind="ExternalOutput")
    out_d32 = nc.dram_tensor("out_d32", [128, RT], F32, kind="ExternalOutput")
    out_pr = nc.dram_tensor("out_pr", [128, RT], F32, kind="ExternalOutput")
    out_cs = nc.dram_tensor("out_cs", [4, 128, 3 * 512], F32, kind="ExternalOutput")

    with ExitStack() as ctx:
        tc = ctx.enter_context(tile.TileContext(nc))
        singles = ctx.enter_context(tc.tile_pool(name="singles", bufs=1))
        sqp = ctx.enter_context(tc.tile_pool(name="sqp", bufs=2))
        estp = ctx.enter_context(tc.tile_pool(name="estp", bufs=3))
        stgp = ctx.enter_context(tc.tile_pool(name="stgp", bufs=2))
        gpool = ctx.enter_context(tc.tile_pool(name="gpool", bufs=2, space="PSUM"))
        tppool = ctx.enter_context(tc.tile_pool(name="tppool", bufs=1, space="PSUM"))
        gridp = ctx.enter_context(tc.tile_pool(name="gridp", bufs=1, space="PSUM"))

        # ---- constants (pre-built, dependency-free) ----
        ident = _ident_t.ap()
        ones_sb = _ones_t.ap()
        ldw_dummy = _ldw_t.ap()
        neg_ap = _neg_t.ap()

        one_ap = nc.const_aps.tensor(1.0, (128, 1))
        # Trigger the natural_log_exp table load right away, overlapping
        # the first z DMA (first call to a new act set costs ~2.7us).
        act_dummy = singles.tile([128, 1], F32)
        nc.scalar.activation(out=act_dummy, in_=one_ap, func=AF.Ln)

        # Wait absorbers for the single-sync-wait walrus.  Each absorb
        # writes a distinct column of a scratch tile so absorbs carry no
        # WAW dependency on each other (which would cost a second wait).
        dve_dummy = singles.tile([1, 64], F32)
        act_scr = singles.tile([128, 64], F32)
        pool_scr = singles.tile([1, 64], F32)
        _absorb_ctr = [0, 0, 0]

        def pe_absorb(dep):
            lw = nc.tensor.ldweights(weights=ldw_dummy)
            add_dep_helper(lw.ins, dep.ins, sync=True,
                           reason="absorb cross-engine wait on PE")

        def act_absorb(dep):
            k = _absorb_ctr[0]
            _absorb_ctr[0] += 1
            a = nc.scalar.mul(act_scr[:, k:k + 1], one_ap, 1.0)
            add_dep_helper(a.ins, dep.ins, sync=True,
                           reason="absorb cross-engine wait on ACT")
            return a

        def dve_absorb(dep):
            k = _absorb_ctr[1]
            _absorb_ctr[1] += 1
            m = nc.vector.memset(dve_dummy[:, k:k + 1], 0.0)
            add_dep_helper(m.ins, dep.ins, sync=True,
                           reason="absorb cross-engine wait on DVE")

        def pool_absorb(dep):
            k = _absorb_ctr[2]
            _absorb_ctr[2] += 1
            m = nc.gpsimd.memset(pool_scr[:, k:k + 1], 0.0)
            add_dep_helper(m.ins, dep.ins, sync=True,
                           reason="absorb cross-engine wait on Pool")

        # ---- persistent SBUF state ----
        z_sb = singles.tile([128, LT, D], F32)
        zn_sb = singles.tile([128, LT, D], BF16)
        znT = singles.tile([128, LT * 128], BF16)
        nrm2 = singles.tile([128, LT], F32)
        lgn = singles.tile([128, LT], F32)
        inv = singles.tile([128, LT], F32)
        d32exp = singles.tile([128, RT * 128], BF16)
        rsparts = singles.tile([128, RT * NCH], F32)
        prod = singles.tile([128, RT, D], F32)
        rs_stage = singles.tile([128, RT], F32)
        d32_stage = singles.tile([128, RT], F32)
        pr_stage = singles.tile([128, RT], F32)

        # gpool slot bookkeeping (bufs=2): exactly one reader is appended
        # per allocation; absorb the reader two allocations back on the PE
        # before reusing its buffer.
        greaders = []

        def new_g(shape, dtype, tag):
            if len(greaders) >= 2:
                pe_absorb(greaders[-2])
            t = gpool.tile(shape, dtype, tag=tag, name=tag)
            greaders.append(None)  # placeholder, fill via set_reader
            return t

        def set_reader(ins):
            # fill the most recent placeholder
            for j in range(len(greaders) - 1, -1, -1):
                if greaders[j] is None:
                    greaders[j] = ins
                    return
            raise AssertionError("no placeholder")

        grid_readers = []

        # z_win arrives host-permuted as [p, t, d] so each partition's DMA
        # lines are contiguous 4KB blocks instead of 512B strided lines
        z_re = z_win[:, :].rearrange("(p t) d -> p t d", p=128)

        # ---- phase 1: load 8 tiles, norms on Pool, inv on ACT, scale on
        # DVE, transpose on PE, copy into znT on DVE ----
        pair_copy = {}

        sq_readers = []
        quad_copy = {}

        def emit_zquad(q):
            sl = slice(q * 4, (q + 1) * 4)
            dma = nc.sync.dma_start(out=z_sb[:, sl, :], in_=z_re[:, sl, :])
            # squares on Pool; absorb the DMA + the sq ring reuse first
            if len(sq_readers) >= 2:
                pool_absorb(sq_readers[-2])
            pool_absorb(dma)
            sq = sqp.tile([128, 4, D], BF16, tag="sq", name="sq")
            nc.gpsimd.tensor_mul(sq, z_sb[:, sl, :], z_sb[:, sl, :])
            rd = nc.vector.tensor_reduce(out=nrm2[:, sl], in_=sq,
                                         axis=AX.X, op=ALU.add)
            sq_readers.append(rd)
            # (no eps clamp: inputs are randn, |z|^2 ~ chi2(128) >> eps)
            # inv = exp(-0.5 * ln(nrm2)) on ACT; per-quad ops land in the
            # lead-in window where ACT is otherwise idle
            nc.scalar.activation(out=lgn[:, sl], in_=nrm2[:, sl], func=AF.Ln)
            iv = nc.scalar.activation(out=inv[:, sl], in_=lgn[:, sl],
                                      func=AF.Exp, scale=-0.5)
            # zn = z * inv (bf16).  Deps: z DMA + iv(ACT); absorb the ACT
            # one so the STT carries a single wait.
            dve_absorb(iv)
            ivb = inv[:, sl]
            ivb = bass.AP(tensor=ivb.tensor, offset=ivb.offset,
                          ap=[ivb.ap[0], ivb.ap[1], [0, D]])
            sc = nc.vector.scalar_tensor_tensor(
                out=zn_sb[:, sl, :], in0=z_sb[:, sl, :], scalar=0.0, in1=ivb,
                op0=ALU.bypass, op1=ALU.mult,
            )
            # dedicated PSUM ring for transposes: its slot-reuse reader is
            # the znT copy (DVE), so the transposes' deps (zn scale + slot)
            # are all DVE-side and merge into a single wait -- no absorbs
            ps = tppool.tile([128, 512], BF16, tag="tp", name="tp")
            # funnel the transposes' DVE deps (previous quad's znT copy =
            # the tp slot reader, and this quad's zn scale) through PE
            # absorbs; the transposes then carry one PE self-wait
            if q > 0:
                pe_absorb(quad_copy[q - 1])
            pe_absorb(sc)
            for t in range(4):
                tt = q * 4 + t
                nc.tensor.transpose(out=ps[:, t * 128:(t + 1) * 128],
                                    in_=zn_sb[:, tt, :], identity=ident)
            cp = nc.vector.tensor_copy(
                out=znT[:, q * 512:(q + 1) * 512], in_=ps)
            quad_copy[q] = cp
            return cp

        # just-in-time PE absorb of the znT quad a chunk needs
        absorbed_q = [-1]

        def need_q(s, ci):
            q = (s + 8 * ci + 7) // 4
            if q > absorbed_q[0]:
                pe_absorb(quad_copy[q])
                absorbed_q[0] = q

        # ---- phase 2 helpers ----
        def emit_strip_mm_exp(i, estrip, reuse_dep=None, pe_cover=None):
            """PE+ACT interleaved per chunk: G chunk matmuls then exp with
            per-chunk row-sum accumulation (combined by a tiny DVE reduce
            at the end)."""
            lhsT = znT[:, i * 128:(i + 1) * 128]
            if reuse_dep is not None:
                # one ACT self-wait >= the 3-back strip's last exp covers
                # every chunk's estrip ring-buffer WAW
                act_absorb(reuse_dep)
            last = None
            for ci in range(NCH):
                off = ci * CHW
                need_q(i, ci)
                gt = new_g([128, CHW], F32, "g")
                first_mm = None
                for c in range(0, CHW, 512):
                    col = i * 128 + off + c
                    mm = nc.tensor.matmul(
                        out=gt[:, c:c + 512],
                        lhsT=lhsT,
                        rhs=znT[:, col:col + 512],
                        start=True, stop=True,
                    )
                    if first_mm is None:
                        first_mm = mm
                        if ci == 1 and pe_cover is not None:
                            # order after the pair's last ones_first matmul:
                            # its direct ACT wait covers this chunk's psum-
                            # slot reader, folding everything into one
                            # PE self-wait
                            add_dep_helper(first_mm.ins, pe_cover.ins,
                                           sync=True,
                                           reason="slot cover via ones")
                a = nc.scalar.activation(
                    out=estrip[:, off:off + CHW], in_=gt, func=AF.Exp,
                    scale=TEMP_INV, bias=neg_ap,
                    accum_out=rsparts[:, i * NCH + ci:i * NCH + ci + 1],
                )
                set_reader(a)
                last = a
            return last

        def ones_mm(grid, estrip, k, e0, e1, start, stop):
            """One ones-matmul: grid slot k += colsums of estrip[:, e0:e1]."""
            p0 = (k % 3) * 32
            f0 = (k // 3) * 512
            return nc.tensor.matmul(
                out=grid[p0:p0 + 1, f0:f0 + (e1 - e0)],
                lhsT=ones_sb,
                rhs=estrip[:, e0:e1],
                start=start, stop=stop, skip_group_check=True,
            )

        def emit_ones_first(grid, estrip):
            """Strip a of a pair: slots 0..7, grid col g = estrip col g+128.
            start=True clears has_written only for the WRITTEN region, so
            every slot's first touch within a pair must be start=True."""
            mm = None
            for k in range(8):
                e0 = 128 + 512 * k
                e1 = min(e0 + 512, 128 + OW)
                mm = ones_mm(grid, estrip, k, e0, e1, start=True,
                             stop=(k == 0))
            return mm

        def emit_ones_second(grid, estrip):
            """Strip b=a+4: slots 1..8 (accumulating onto strip a), grid
            col g = estrip col g-384.  Slot 7's tail [384:512) and slot 8
            are first-touch (start=True); slot 7 is split accordingly."""
            mm = None
            for k in range(1, 7):
                e0 = 512 * k - 384
                mm = ones_mm(grid, estrip, k, e0, e0 + 512, start=False,
                             stop=True)
            # slot 7: [0:384) accumulates, [384:512) is fresh
            ones_mm(grid, estrip, 7, 3200, 3584, start=False, stop=True)
            ones_mm7 = nc.tensor.matmul(
                out=grid[32:33, 1408:1536],
                lhsT=ones_sb,
                rhs=estrip[:, 3584:3712],
                start=True, stop=True, skip_group_check=True,
            )
            # slot 8: fresh [0:384)
            mm = ones_mm(grid, estrip, 8, 3712, 4096, start=True, stop=True)
            return mm

        stg_dmas = []

        def emit_grid_export(pi, grid, last_ones):
            if len(stg_dmas) >= 2:
                # staging-buffer reuse (old export DMA) and the fresh ones
                # matmuls both absorbed on DVE; the copy self-waits once
                dve_absorb(stg_dmas[-2])
                dve_absorb(last_ones)
            stg = stgp.tile([128, 3 * 512], F32, tag="stg", name="stg")
            cp = nc.vector.tensor_copy(out=stg, in_=grid)
            grid_readers.append(cp)
            d = nc.gpsimd.dma_start(out=out_cs[pi, :, :], in_=stg[:, :])
            stg_dmas.append(d)

        # ---- emission ----
        # strip 0's chunk ci only needs z quads 2ci..2ci+1: interleave its
        # chunks with the phase-1 quads so no engine queue is head-of-line
        # blocked behind later quads' phase-1 work.
        emit_zquad(0)
        emit_zquad(1)

        es_of = {}
        exp_of = {}
        grid = None
        grid_pi = -1
        for k, s in enumerate(STRIP_ORDER):
            estrip = estp.tile([128, SW], BF16, tag="es", name="es")
            es_of[s] = estrip
            if k == 0:
                lhsT = znT[:, 0:128]
                for ci in range(NCH):
                    need_q(0, ci)
                    gt = new_g([128, CHW], F32, "g")
                    for c in range(0, CHW, 512):
                        col = ci * CHW + c
                        nc.tensor.matmul(
                            out=gt[:, c:c + 512], lhsT=lhsT,
                            rhs=znT[:, col:col + 512],
                            start=True, stop=True,
                        )
                    a = nc.scalar.activation(
                        out=estrip[:, ci * CHW:(ci + 1) * CHW], in_=gt,
                        func=AF.Exp, scale=TEMP_INV, bias=neg_ap,
                        accum_out=rsparts[:, ci:ci + 1],
                    )
                    set_reader(a)
                    exp_of[s] = a
                    emit_zquad(2 * ci + 2)
                    emit_zquad(2 * ci + 3)
            else:
                if k % 2 == 1:
                    # strip b of pair pi=(k-1)//2: open the pair's grid and
                    # run strip a's ones BEFORE strip b's chunks -- the ones
                    # matmuls carry direct ACT waits on strip a's exps,
                    # which also covers the chunk matmuls' psum-slot deps
                    pi = (k - 1) // 2
                    if grid is not None:
                        pe_absorb(grid_readers[-1])
                    grid = gridp.tile([128, 3 * 512], F32,
                                      tag="grid", name="grid")
                    ones_last = emit_ones_first(grid, es_of[STRIP_ORDER[k - 1]])
                    grid_pi = pi
                else:
                    ones_last = None
                exp_of[s] = emit_strip_mm_exp(
                    s, estrip,
                    reuse_dep=(exp_of[STRIP_ORDER[k - 3]] if k >= 3 else None),
                    pe_cover=ones_last,
                )
            if k % 2 == 0 and k > 0:
                # strip b's ones of the previous pair + grid export
                prev_b = STRIP_ORDER[k - 1]
                lmm = emit_ones_second(grid, es_of[prev_b])
                emit_grid_export(grid_pi, grid, lmm)

        lmm = emit_ones_second(grid, es_of[STRIP_ORDER[-1]])
        emit_grid_export(grid_pi, grid, lmm)
        # combine per-chunk row-sum partials
        nc.vector.tensor_reduce(
            out=rs_stage, in_=rsparts.rearrange("p (s c) -> p s c", s=RT),
            axis=AX.X, op=ALU.add)

        # ---- d32 blocks (i, i+32): row sums only ----
        g32 = new_g([128, 1024], F32, "g")
        for i in range(RT):
            nc.tensor.matmul(
                out=g32[:, i * 128:(i + 1) * 128],
                lhsT=znT[:, i * 128:(i + 1) * 128],
                rhs=znT[:, (i + 32) * 128:(i + 33) * 128],
                start=True, stop=True,
            )
        a32 = nc.scalar.activation(
            out=d32exp[:, :], in_=g32, func=AF.Exp,
            scale=TEMP_INV, bias=neg_ap,
        )
        set_reader(a32)
        nc.vector.tensor_reduce(
            out=d32_stage, in_=d32exp.rearrange("p (t d) -> p t d", t=RT),
            axis=AX.X, op=ALU.add)

        # ---- positives: praw_i = rowdot(zn_i, zn_{i+32}) ----
        nc.vector.scalar_tensor_tensor(
            out=prod, in0=zn_sb[:, 0:RT, :], scalar=0.0,
            in1=zn_sb[:, 32:32 + RT, :], op0=ALU.bypass, op1=ALU.mult,
        )
        nc.vector.tensor_reduce(out=pr_stage, in_=prod, axis=AX.X, op=ALU.add)

        # ---- exports ----
        nc.gpsimd.dma_start(out=out_rs[:, :], in_=rs_stage)
        nc.gpsimd.dma_start(out=out_d32[:, :], in_=d32_stage)
        nc.gpsimd.dma_start(out=out_pr[:, :], in_=pr_stage)

    return nc


_NC_CACHE: dict = {}


def _get_nc() -> bass.Bass:
    if "nc" not in _NC_CACHE:
        _NC_CACHE["nc"] = build_kernel()
    return _NC_CACHE["nc"]


def make_in_maps(z1: np.ndarray, z2: np.ndarray):
    z = np.ascontiguousarray(
        np.concatenate([z1, z2], axis=0), dtype=np.float32
    )
    in_maps = []
    # [p, t, d] layout: row p*LT + t holds logical row rot + t*128 + p,
    # making each partition's DMA source contiguous
    p_idx = np.repeat(np.arange(128), LT)
    t_idx = np.tile(np.arange(LT), 128)
    for c in range(N_CORES):
        rows = (c * RT * 128 + t_idx * 128 + p_idx) % M2
        in_maps.append({"z_win": np.ascontiguousarray(z[rows])})
    return in_maps


def finish(results) -> np.ndarray:
    S = np.zeros(M2, dtype=np.float64)
    praw = np.zeros(M2, dtype=np.float64)
    for c, r in enumerate(results):
        rs = r["out_rs"].astype(np.float64)
        d32 = r["out_d32"].astype(np.float64)
        pr = r["out_pr"].astype(np.float64)
        cs = r["out_cs"].astype(np.float64)
        for i in range(RT):
            lo = (RT * c + i) * 128
            S[lo:lo + 128] += rs[:, i] + d32[:, i]
            praw[lo:lo + 128] = pr[:, i]
        for pi in range(4):
            a = pi  # pair = (strips a, a+4), grid base col = (a+1)*128
            vec = np.empty(GW, dtype=np.float64)
            for k in range(9):
                vec[k * 512:(k + 1) * 512] = cs[pi, (k % 3) * 32,
                                                (k // 3) * 512:(k // 3 + 1) * 512]
            vec = vec[:GV]
            start = ((RT * c + a + 1) * 128) % M2
            end = start + GV
            if end <= M2:
                S[start:end] += vec
            else:
                kk = M2 - start
                S[start:] += vec[:kk]
                S[:GV - kk] += vec[kk:]
    pos = 2.0 * TEMP_INV * praw
    # S includes the diagonal self-term exp(10*|zn_r|^2 - 10) ~ 1
    den = np.exp(pos - LSE_SHIFT) + S - 1.0
    L = LSE_SHIFT + np.log(den) - pos
    return np.float32(L.sum() / (float(M2) * float(M2)))


def kernel(z1: np.ndarray, z2: np.ndarray, **run_kwargs) -> np.ndarray:
    nc = _get_nc()
    in_maps = make_in_maps(z1, z2)
    res = run_bass_kernel_spmd(nc, in_maps, core_ids=list(range(N_CORES)),
                               **run_kwargs)
    out = finish(res.results)
    kernel.last_results = res
    return out
